# revision 38
# baseline (speedup 1.0000x reference)
"""Trainium2 Bass kernel for KMGCN (2x GCNConv + global mean pool + FC), 8 cores.

Single fused launch. Nodes are partitioned contiguously across 8 cores
(6250 each, dst-sharded). Host ships only the x shard plus compact edge
index/weight streams; ALL gathers run on device:

  - AllGather x shards -> full 50000x128 table in device DRAM
  - per dst-tile, gpsimd.dma_gather fetches x[src] rows (table split at
    32768 to fit signed int16 gather indices)
  - one-hot scatter matmuls (PSUM accumulation) do the sym-normalized
    aggregation; W1/W2 transforms fused per tile
  - h2pre written back to DRAM, AllGather -> layer-2 table, same
    gather+aggregate pass, then matmul pooling, AllReduce, FC.
"""

import ml_dtypes
import numpy as np
import concourse.bass as bass
import concourse.bacc as bacc
import concourse.tile as tile
import concourse.mybir as mybir
from concourse.bass_utils import run_bass_kernel_spmd

NCORES = 8
F32 = mybir.dt.float32
BF16 = mybir.dt.bfloat16
I16 = mybir.dt.int16
TSPLIT = 32768
GCAP = 6  # max chunks (x128 idxs) per dma_gather call; 1024 idxs is the HW limit, keep margin
TDT = BF16  # dtype of the gather tables (x shard, h2pre)

_cache = {}
last_result = None
exec_wall = [0.0]


def _plan(src, dst, n_nodes):
    """Per-core edge streams chunked by (dst_tile, table_half), padded so all
    cores share one program."""
    npc = n_nodes // NCORES
    ntile = (npc + 127) // 128
    deg = np.bincount(dst, minlength=n_nodes).astype(np.float32) + 1.0
    dinv = 1.0 / np.sqrt(deg)
    a_src = np.concatenate([src, np.arange(n_nodes, dtype=src.dtype)])
    a_dst = np.concatenate([dst, np.arange(n_nodes, dtype=src.dtype)])
    a_w = (dinv[a_src] * dinv[a_dst]).astype(np.float32)

    per_core = []
    cnt = np.zeros((NCORES, ntile, 2), np.int64)
    for c in range(NCORES):
        m = (a_dst >= c * npc) & (a_dst < (c + 1) * npc)
        es, ed, ew = a_src[m], a_dst[m] - c * npc, a_w[m]
        tl = ed >> 7
        hf = (es >= TSPLIT).astype(np.int64)
        order = np.lexsort((hf, tl))
        es, ed, ew, tl, hf = es[order], ed[order], ew[order], tl[order], hf[order]
        per_core.append((es, ed, ew, tl, hf))
        for t in range(ntile):
            mm = tl == t
            cnt[c, t, 0] = int((mm & (hf == 0)).sum())
            cnt[c, t, 1] = int((mm & (hf == 1)).sum())
    cpt = np.ceil(cnt / 128.0).astype(np.int64).max(axis=0)  # [ntile, 2]
    cpt = np.maximum(cpt, 1)
    nch = int(cpt.sum())

    cores = []
    for c in range(NCORES):
        es, ed, ew, tl, hf = per_core[c]
        gidx = np.zeros(nch * 128, np.int16)
        sd = np.zeros(nch * 128, np.float32)
        sw = np.zeros(nch * 128, np.float32)
        pos = 0
        for t in range(ntile):
            for h in range(2):
                mm = (tl == t) & (hf == h)
                n = int(mm.sum())
                gidx[pos : pos + n] = (es[mm] - (TSPLIT if h else 0)).astype(np.int16)
                sd[pos : pos + n] = (ed[mm] - t * 128).astype(np.float32)
                sw[pos : pos + n] = ew[mm]
                pos += int(cpt[t, h]) * 128
        iw = gidx.reshape(-1, 16).T  # [16, nch*8] 16-wrapped
        cores.append((np.ascontiguousarray(iw),
                      np.ascontiguousarray(sd.reshape(nch, 128).T.astype(ml_dtypes.bfloat16)),
                      np.ascontiguousarray(sw.reshape(nch, 128).T.astype(ml_dtypes.bfloat16))))
    return dict(npc=npc, ntile=ntile, cpt=cpt, nch=nch, cores=cores)


def _build(meta, n_nodes, in_dim, hid, oh, n_graphs, stage=3):
    npc, ntile, cpt, nch = meta["npc"], meta["ntile"], meta["cpt"], meta["nch"]
    CAM, CBM = int(cpt[:, 0].max()), int(cpt[:, 1].max())
    nc = bacc.Bacc("TRN2", target_bir_lowering=False, debug=False,
                   num_devices=NCORES)
    t_xs = nc.dram_tensor("xs", [npc, in_dim], TDT, kind="ExternalInput")
    t_gidx = nc.dram_tensor("gidx", [16, nch * 8], I16, kind="ExternalInput")
    t_sd = nc.dram_tensor("sd", [128, nch], BF16, kind="ExternalInput")
    t_sw = nc.dram_tensor("sw", [128, nch], BF16, kind="ExternalInput")
    t_w1 = nc.dram_tensor("w1", [in_dim, hid], F32, kind="ExternalInput")
    t_b1 = nc.dram_tensor("b1c", [128, hid // 128], F32, kind="ExternalInput")
    t_w2 = nc.dram_tensor("w2", [hid, oh], F32, kind="ExternalInput")
    t_b2r = nc.dram_tensor("b2r", [128, oh], F32, kind="ExternalInput")
    t_bt = nc.dram_tensor("batchv", [128, ntile], F32, kind="ExternalInput")
    t_icnt = nc.dram_tensor("invcnt", [n_graphs, 1], F32, kind="ExternalInput")
    t_wfc = nc.dram_tensor("wfc", [oh, 8], F32, kind="ExternalInput")
    t_bfc = nc.dram_tensor("bfc", [n_graphs, 8], F32, kind="ExternalInput")
    t_iota = nc.dram_tensor("iota", [128, 128], F32, kind="ExternalInput")
    t_eye = nc.dram_tensor("eye", [128, 128], F32, kind="ExternalInput")
    t_out = nc.dram_tensor("out", [n_graphs, 8], F32, kind="ExternalOutput")
    t_dbg = (nc.dram_tensor("dbg", [ntile * 128, oh], F32, kind="ExternalOutput")
             if stage in (1, 2) else None)

    nh = hid // 128  # 2
    with tile.TileContext(nc) as tc:
        with (
            tc.tile_pool(name="consts", bufs=1) as cp,
            tc.tile_pool(name="dram", bufs=1, space="DRAM") as dp,
        ):
            iota = cp.tile([128, 128], F32)
            eye = cp.tile([128, 128], F32)
            gidx = cp.tile([128, nch * 8], I16)
            sdt = cp.tile([128, nch], BF16)
            swt = cp.tile([128, nch], BF16)
            w1 = cp.tile([in_dim, hid], F32)
            b1c = cp.tile([128, nh], F32)
            w2h = [cp.tile([128, oh], F32, name=f"w2h{h}") for h in range(nh)]
            b2r = cp.tile([128, oh], F32)
            btv = cp.tile([128, ntile], F32)
            icnt = cp.tile([n_graphs, 1], F32)
            wfc = cp.tile([oh, 8], F32)
            bfc = cp.tile([n_graphs, 8], F32)
            for tt, dt in ((iota, t_iota), (eye, t_eye),
                           (sdt, t_sd), (swt, t_sw), (w1, t_w1), (b1c, t_b1),
                           (b2r, t_b2r), (btv, t_bt),
                           (icnt, t_icnt), (wfc, t_wfc), (bfc, t_bfc)):
                nc.sync.dma_start(out=tt[:, :], in_=dt[:, :])
            for k in range(8):  # replicate 16-row idx wrap across 128 partitions
                nc.sync.dma_start(out=gidx[16 * k : 16 * (k + 1), :],
                                  in_=t_gidx[:, :])
            for h in range(nh):
                nc.sync.dma_start(out=w2h[h][:, :],
                                  in_=t_w2[h * 128 : (h + 1) * 128, :])

            # stage shard into internal DRAM (collectives can't read IO), gather full x
            xl = dp.tile([npc, in_dim], TDT)
            nc.sync.dma_start(out=xl[:, :], in_=t_xs[:, :])
            xfull = dp.tile([n_nodes, in_dim], TDT, addr_space="Shared")
            nc.gpsimd.collective_compute(
                "AllGather", mybir.AluOpType.bypass,
                replica_groups=[list(range(NCORES))],
                ins=[xl[:, :].opt()], outs=[xfull[:, :].opt()])
            h2l = dp.tile([npc, oh], TDT)

            # ---------------- layer 1 ----------------
            with (
                tc.tile_pool(name="gp1", bufs=2) as gp,
                tc.tile_pool(name="sp1", bufs=2) as sp,
                tc.tile_pool(name="st1", bufs=3) as stp,
                tc.tile_pool(name="ps_agg1", bufs=2, space="PSUM") as ps_agg,
                tc.tile_pool(name="ps_h1", bufs=2, space="PSUM") as ps_h,
                tc.tile_pool(name="ps_h2", bufs=2, space="PSUM") as ps_h2p,
                tc.tile_pool(name="ps_tr1", bufs=2, space="PSUM") as ps_tr,
            ):
                ch = 0
                for t in range(ntile):
                    ca, cb = int(cpt[t, 0]), int(cpt[t, 1])
                    gA = gp.tile([128, CAM, 128], TDT, tag="gA")
                    gB = gp.tile([128, CBM, 128], TDT, tag="gB")
                    for o in range(0, ca, GCAP):
                        k = min(GCAP, ca - o)
                        nc.gpsimd.dma_gather(
                            gA[:, o : o + k, :], xfull[0:TSPLIT, :],
                            gidx[:, (ch + o) * 8 : (ch + o + k) * 8],
                            k * 128, k * 128, in_dim)
                    for o in range(0, cb, GCAP):
                        k = min(GCAP, cb - o)
                        nc.gpsimd.dma_gather(
                            gB[:, o : o + k, :], xfull[TSPLIT:n_nodes, :],
                            gidx[:, (ch + ca + o) * 8 : (ch + ca + o + k) * 8],
                            k * 128, k * 128, in_dim)
                    pt = ps_agg.tile([128, 128], F32, tag="agg")
                    nck = ca + cb
                    sg = sp.tile([128, CAM + CBM, 128], TDT, tag="s")
                    iota_b = iota[:, :].unsqueeze(1).broadcast_to([128, nck, 128])
                    sd_b = sdt[:, ch : ch + nck].unsqueeze(2).broadcast_to(
                        [128, nck, 128])
                    sw_b = swt[:, ch : ch + nck].unsqueeze(2).broadcast_to(
                        [128, nck, 128])
                    nc.vector.tensor_tensor(out=sg[:, 0:nck, :], in0=iota_b,
                                            in1=sd_b, op=mybir.AluOpType.is_equal)
                    nc.vector.tensor_tensor(out=sg[:, 0:nck, :], in0=sg[:, 0:nck, :],
                                            in1=sw_b, op=mybir.AluOpType.mult)
                    for j in range(nck):
                        g = gA[:, j, :] if j < ca else gB[:, j - ca, :]
                        # feat-major agg^T: out[feat, dstlane]
                        nc.tensor.matmul(pt[:, :], lhsT=g, rhs=sg[:, j, :],
                                         start=(j == 0), stop=(j == nck - 1))
                    ch += nck
                    aggs = stp.tile([128, 128], F32, tag="aggs")
                    nc.vector.tensor_copy(aggs[:, :], pt[:, :])
                    # h1^T = relu(W1^T agg + b1), two 128-halves
                    ph = ps_h.tile([128, hid], F32, tag="ph")
                    h1s = stp.tile([128, hid], F32, tag="h1s")
                    for h in range(nh):
                        nc.tensor.matmul(ph[:, h * 128 : (h + 1) * 128],
                                         lhsT=w1[:, h * 128 : (h + 1) * 128],
                                         rhs=aggs[:, :], start=True, stop=True)
                        nc.scalar.activation(
                            out=h1s[:, h * 128 : (h + 1) * 128],
                            in_=ph[:, h * 128 : (h + 1) * 128],
                            func=mybir.ActivationFunctionType.Relu,
                            bias=b1c[:, h : h + 1], scale=1.0)
                    # h2pre^T = W2^T h1
                    p2 = ps_h2p.tile([128, oh], F32, tag="p2")
                    for h in range(nh):
                        nc.tensor.matmul(p2[:, :], lhsT=w2h[h][:, :],
                                         rhs=h1s[:, h * 128 : (h + 1) * 128],
                                         start=(h == 0), stop=(h == nh - 1))
                    hp = stp.tile([128, oh], F32, tag="hp")
                    nc.vector.tensor_copy(hp[:, :], p2[:, :])
                    ptr = ps_tr.tile([128, 128], F32, tag="tr")
                    nc.tensor.transpose(ptr[:, :], hp[:, :], eye[:, :])
                    ro = stp.tile([128, 128], TDT, tag="ro")
                    nc.vector.tensor_copy(ro[:, :], ptr[:, :])
                    rows = min(128, npc - t * 128)
                    nc.sync.dma_start(out=h2l[t * 128 : t * 128 + rows, :],
                                      in_=ro[0:rows, :])
                    if stage == 1:
                        nc.sync.dma_start(
                            out=t_dbg[t * 128 : (t + 1) * 128, :], in_=ro[:, :])

            if stage < 3:
                with tc.tile_pool(name="zz", bufs=1) as zp:
                    zz = zp.tile([n_graphs, 8], F32)
                    nc.vector.tensor_scalar(out=zz[:, :], in0=bfc[:, :], scalar1=0.0,
                                            scalar2=None, op0=mybir.AluOpType.mult)
                    nc.sync.dma_start(out=t_out[:, :], in_=zz[:, :])

            if stage >= 2:
                _build_l2_section(nc, tc, meta, n_nodes, oh, n_graphs, stage,
                                  dp, cp, locals())
    nc.compile()
    return nc


def _build_l2_section(nc, tc, meta, n_nodes, oh, n_graphs, stage, dp, cp, env):
    npc, ntile, cpt, nch = meta["npc"], meta["ntile"], meta["cpt"], meta["nch"]
    CAM, CBM = int(cpt[:, 0].max()), int(cpt[:, 1].max())
    iota, gidx, sdt, swt = env["iota"], env["gidx"], env["sdt"], env["swt"]
    b2r, btv, icnt, wfc, bfc = (env["b2r"], env["btv"], env["icnt"],
                                env["wfc"], env["bfc"])
    h2l, t_out, t_dbg = env["h2l"], env["t_out"], env["t_dbg"]

    h2full = dp.tile([n_nodes, oh], TDT, addr_space="Shared")
    nc.gpsimd.collective_compute(
        "AllGather", mybir.AluOpType.bypass,
        replica_groups=[list(range(NCORES))],
        ins=[h2l[:, :].opt()], outs=[h2full[:, :].opt()])

    # ---------------- layer 2 + pool ----------------
    with (
                tc.tile_pool(name="gp2", bufs=2) as gp,
                tc.tile_pool(name="sp2", bufs=2) as sp,
                tc.tile_pool(name="st2", bufs=3) as stp,
                tc.tile_pool(name="ps_agg2", bufs=2, space="PSUM") as ps_agg,
                tc.tile_pool(name="ps_pool", bufs=1, space="PSUM") as ps_pool,
                tc.tile_pool(name="ps_fc", bufs=1, space="PSUM") as ps_fc,
            ):
                ppool = ps_pool.tile([128, n_graphs], F32)
                ch = 0
                for t in range(ntile):
                    ca, cb = int(cpt[t, 0]), int(cpt[t, 1])
                    gA = gp.tile([128, CAM, 128], TDT, tag="gA")
                    gB = gp.tile([128, CBM, 128], TDT, tag="gB")
                    for o in range(0, ca, GCAP):
                        k = min(GCAP, ca - o)
                        nc.gpsimd.dma_gather(
                            gA[:, o : o + k, :], h2full[0:TSPLIT, :],
                            gidx[:, (ch + o) * 8 : (ch + o + k) * 8],
                            k * 128, k * 128, oh)
                    for o in range(0, cb, GCAP):
                        k = min(GCAP, cb - o)
                        nc.gpsimd.dma_gather(
                            gB[:, o : o + k, :], h2full[TSPLIT:n_nodes, :],
                            gidx[:, (ch + ca + o) * 8 : (ch + ca + o + k) * 8],
                            k * 128, k * 128, oh)
                    pt = ps_agg.tile([128, 128], F32, tag="agg")
                    nck = ca + cb
                    sg = sp.tile([128, CAM + CBM, 128], TDT, tag="s")
                    iota_b = iota[:, :].unsqueeze(1).broadcast_to([128, nck, 128])
                    sd_b = sdt[:, ch : ch + nck].unsqueeze(2).broadcast_to(
                        [128, nck, 128])
                    sw_b = swt[:, ch : ch + nck].unsqueeze(2).broadcast_to(
                        [128, nck, 128])
                    nc.vector.tensor_tensor(out=sg[:, 0:nck, :], in0=iota_b,
                                            in1=sd_b, op=mybir.AluOpType.is_equal)
                    nc.vector.tensor_tensor(out=sg[:, 0:nck, :], in0=sg[:, 0:nck, :],
                                            in1=sw_b, op=mybir.AluOpType.mult)
                    for j in range(nck):
                        g = gA[:, j, :] if j < ca else gB[:, j - ca, :]
                        # node-major: out[dstlane, feat]
                        nc.tensor.matmul(pt[:, :], lhsT=sg[:, j, :], rhs=g,
                                         start=(j == 0), stop=(j == nck - 1))
                    ch += nck
                    h2 = stp.tile([128, oh], F32, tag="h2")
                    nc.vector.tensor_tensor(out=h2[:, :], in0=pt[:, :],
                                            in1=b2r[:, :], op=mybir.AluOpType.add)
                    nc.vector.tensor_scalar(
                        out=h2[:, :], in0=h2[:, :], scalar1=0.0, scalar2=None,
                        op0=mybir.AluOpType.max)
                    eq = sp.tile([128, n_graphs], F32, tag="eq")
                    nc.vector.tensor_scalar(
                        out=eq[:, :], in0=iota[:, 0:n_graphs],
                        scalar1=btv[:, t : t + 1], scalar2=None,
                        op0=mybir.AluOpType.is_equal)
                    nc.tensor.matmul(ppool[:, :], lhsT=h2[:, :], rhs=eq[:, :],
                                     start=(t == 0), stop=(t == ntile - 1))
                    if stage == 2:
                        nc.sync.dma_start(
                            out=t_dbg[t * 128 : (t + 1) * 128, :], in_=h2[:, :])

                if stage == 2:
                    return
                # per-core partial FC output; host sums partials and adds bfc
                pooled = stp.tile([128, n_graphs], F32, tag="pooled")
                nc.vector.tensor_copy(pooled[:, :], ppool[:, :])
                pfc = ps_fc.tile([n_graphs, 8], F32)
                nc.tensor.matmul(pfc[:, :], lhsT=pooled[:, :], rhs=wfc[:, :],
                                 start=True, stop=True)
                osb = stp.tile([n_graphs, 8], F32, tag="osb")
                nc.vector.tensor_scalar(
                    out=osb[:, :], in0=pfc[:, :], scalar1=icnt[:, 0:1],
                    scalar2=None, op0=mybir.AluOpType.mult)
                nc.sync.dma_start(out=t_out[:, :], in_=osb[:, :])
    nc.compile()
    return nc


def kernel(x, src, dst, batch, W1, b1, W2, b2, Wfc, bfc):
    global last_result
    x = np.asarray(x, np.float32)
    src = np.asarray(src, np.int64)
    dst = np.asarray(dst, np.int64)
    batch = np.asarray(batch, np.int64)
    W1, b1v, W2, b2v, Wfc, bfcv = (np.asarray(a, np.float32)
                                   for a in (W1, b1, W2, b2, Wfc, bfc))
    n, in_dim = x.shape
    hid = W1.shape[1]
    oh = W2.shape[1]
    ng = 64
    odim = Wfc.shape[1]

    meta = _plan(src, dst, n)
    npc, ntile, nch = meta["npc"], meta["ntile"], meta["nch"]

    key = (n, in_dim, hid, oh, meta["cpt"].tobytes())
    if key not in _cache:
        _cache[key] = _build(meta, n, in_dim, hid, oh, ng)
    nc = _cache[key]

    iota = np.tile(np.arange(128, dtype=np.float32), (128, 1))
    eye = np.eye(128, dtype=np.float32)
    cntg = np.maximum(np.bincount(batch, minlength=ng).astype(np.float32), 1.0)
    icnt = (1.0 / cntg).reshape(ng, 1)
    b1c = np.ascontiguousarray(b1v.reshape(hid // 128, 128).T)
    b2r = np.tile(b2v.reshape(1, oh), (128, 1)).astype(np.float32)
    wfc8 = np.zeros((oh, 8), np.float32)
    wfc8[:, :odim] = Wfc
    bfc8 = np.zeros((ng, 8), np.float32)
    bfc8[:, :odim] = bfcv.reshape(1, odim)

    ins = []
    for c in range(NCORES):
        iw, sdv, swv = meta["cores"][c]
        bl = np.full(ntile * 128, -1.0, np.float32)
        bl[:npc] = batch[c * npc : (c + 1) * npc].astype(np.float32)
        ins.append({
            "xs": x[c * npc : (c + 1) * npc].astype(ml_dtypes.bfloat16)
                  if TDT == BF16 else x[c * npc : (c + 1) * npc],
            "gidx": iw, "sd": sdv, "sw": swv,
            "w1": W1, "b1c": b1c, "w2": W2, "b2r": b2r,
            "batchv": np.ascontiguousarray(bl.reshape(ntile, 128).T),
            "invcnt": icnt, "wfc": wfc8, "bfc": bfc8,
            "iota": iota, "eye": eye,
        })
    import time as _t
    _s = _t.time()
    r = run_bass_kernel_spmd(nc, ins, core_ids=list(range(NCORES)))
    exec_wall[0] = _t.time() - _s
    last_result = r
    out = np.zeros((ng, 8), np.float32)
    for c in range(NCORES):
        out += np.asarray(r.results[c]["out"], np.float32)
    out += bfc8
    return np.ascontiguousarray(out[:ng, :odim])


# revision 39
# speedup vs baseline: 1.5380x; 1.5380x over previous
"""Trainium2 Bass kernel for KMGCN (2x GCNConv + global mean pool + FC), 8 cores.

Single fused launch. Nodes are partitioned contiguously across 8 cores
(6250 each, dst-sharded). Host ships only the x shard plus compact edge
index/weight streams; ALL gathers run on device:

  - AllGather x shards -> full 50000x128 table in device DRAM
  - per dst-tile, gpsimd.dma_gather fetches x[src] rows (table split at
    32768 to fit signed int16 gather indices)
  - one-hot scatter matmuls (PSUM accumulation) do the sym-normalized
    aggregation; W1/W2 transforms fused per tile
  - h2pre written back to DRAM, AllGather -> layer-2 table, same
    gather+aggregate pass, then matmul pooling, AllReduce, FC.
"""

import ml_dtypes
import numpy as np
import concourse.bass as bass
import concourse.bacc as bacc
import concourse.tile as tile
import concourse.mybir as mybir
from concourse.bass_utils import run_bass_kernel_spmd

NCORES = 8
F32 = mybir.dt.float32
BF16 = mybir.dt.bfloat16
I16 = mybir.dt.int16
TSPLIT = 32768
GCAP = 6  # max chunks (x128 idxs) per dma_gather call; 1024 idxs is the HW limit, keep margin
TDT = BF16  # dtype of the gather tables (x shard, h2pre)

_cache = {}
last_result = None
exec_wall = [0.0]


def _plan(src, dst, n_nodes):
    """Per-core edge streams chunked by (dst_tile, table_half), padded so all
    cores share one program."""
    npc = n_nodes // NCORES
    ntile = (npc + 127) // 128
    deg = np.bincount(dst, minlength=n_nodes).astype(np.float32) + 1.0
    dinv = 1.0 / np.sqrt(deg)
    a_src = np.concatenate([src, np.arange(n_nodes, dtype=src.dtype)])
    a_dst = np.concatenate([dst, np.arange(n_nodes, dtype=src.dtype)])
    a_w = (dinv[a_src] * dinv[a_dst]).astype(np.float32)

    per_core = []
    cnt = np.zeros((NCORES, ntile, 2), np.int64)
    for c in range(NCORES):
        m = (a_dst >= c * npc) & (a_dst < (c + 1) * npc)
        es, ed, ew = a_src[m], a_dst[m] - c * npc, a_w[m]
        tl = ed >> 7
        hf = (es >= TSPLIT).astype(np.int64)
        order = np.lexsort((hf, tl))
        es, ed, ew, tl, hf = es[order], ed[order], ew[order], tl[order], hf[order]
        per_core.append((es, ed, ew, tl, hf))
        for t in range(ntile):
            mm = tl == t
            cnt[c, t, 0] = int((mm & (hf == 0)).sum())
            cnt[c, t, 1] = int((mm & (hf == 1)).sum())
    cpt = np.ceil(cnt / 128.0).astype(np.int64).max(axis=0)  # [ntile, 2]
    cpt = np.maximum(cpt, 1)
    nch = int(cpt.sum())

    cores = []
    for c in range(NCORES):
        es, ed, ew, tl, hf = per_core[c]
        gidx = np.zeros(nch * 128, np.int16)
        sd = np.zeros(nch * 128, np.float32)
        sw = np.zeros(nch * 128, np.float32)
        pos = 0
        for t in range(ntile):
            for h in range(2):
                mm = (tl == t) & (hf == h)
                n = int(mm.sum())
                gidx[pos : pos + n] = (es[mm] - (TSPLIT if h else 0)).astype(np.int16)
                sd[pos : pos + n] = (ed[mm] - t * 128).astype(np.float32)
                sw[pos : pos + n] = ew[mm]
                pos += int(cpt[t, h]) * 128
        iw = gidx.reshape(-1, 16).T  # [16, nch*8] 16-wrapped
        cores.append((np.ascontiguousarray(iw),
                      np.ascontiguousarray(sd.reshape(nch, 128).T.astype(ml_dtypes.bfloat16)),
                      np.ascontiguousarray(sw.reshape(nch, 128).T.astype(ml_dtypes.bfloat16))))
    return dict(npc=npc, ntile=ntile, cpt=cpt, nch=nch, cores=cores)


def _build(meta, n_nodes, in_dim, hid, oh, n_graphs, stage=3):
    npc, ntile, cpt, nch = meta["npc"], meta["ntile"], meta["cpt"], meta["nch"]
    CAM, CBM = int(cpt[:, 0].max()), int(cpt[:, 1].max())
    nc = bacc.Bacc("TRN2", target_bir_lowering=False, debug=False,
                   num_devices=NCORES)
    t_xs = nc.dram_tensor("xs", [npc, in_dim], TDT, kind="ExternalInput")
    t_gidx = nc.dram_tensor("gidx", [16, nch * 8], I16, kind="ExternalInput")
    t_sd = nc.dram_tensor("sd", [128, nch], BF16, kind="ExternalInput")
    t_sw = nc.dram_tensor("sw", [128, nch], BF16, kind="ExternalInput")
    t_w1 = nc.dram_tensor("w1", [in_dim, hid], F32, kind="ExternalInput")
    t_b1 = nc.dram_tensor("b1c", [128, hid // 128], F32, kind="ExternalInput")
    t_w2 = nc.dram_tensor("w2", [hid, oh], F32, kind="ExternalInput")
    t_b2r = nc.dram_tensor("b2r", [128, oh], F32, kind="ExternalInput")
    t_bt = nc.dram_tensor("batchv", [128, ntile], F32, kind="ExternalInput")
    t_icnt = nc.dram_tensor("invcnt", [n_graphs, 1], F32, kind="ExternalInput")
    t_wfc = nc.dram_tensor("wfc", [oh, 8], F32, kind="ExternalInput")
    t_bfc = nc.dram_tensor("bfc", [n_graphs, 8], F32, kind="ExternalInput")
    t_iota = nc.dram_tensor("iota", [128, 128], F32, kind="ExternalInput")
    t_eye = nc.dram_tensor("eye", [128, 128], F32, kind="ExternalInput")
    t_out = nc.dram_tensor("out", [n_graphs, 8], F32, kind="ExternalOutput")
    t_dbg = (nc.dram_tensor("dbg", [ntile * 128, oh], F32, kind="ExternalOutput")
             if stage in (1, 2) else None)

    nh = hid // 128  # 2
    with tile.TileContext(nc) as tc:
        with (
            tc.tile_pool(name="consts", bufs=1) as cp,
            tc.tile_pool(name="dram", bufs=1, space="DRAM") as dp,
        ):
            iota = cp.tile([128, 128], F32)
            eye = cp.tile([128, 128], F32)
            gidx = cp.tile([128, nch * 8], I16)
            sdt = cp.tile([128, nch], BF16)
            swt = cp.tile([128, nch], BF16)
            w1 = cp.tile([in_dim, hid], F32)
            b1c = cp.tile([128, nh], F32)
            w2h = [cp.tile([128, oh], F32, name=f"w2h{h}") for h in range(nh)]
            b2r = cp.tile([128, oh], F32)
            btv = cp.tile([128, ntile], F32)
            icnt = cp.tile([n_graphs, 1], F32)
            wfc = cp.tile([oh, 8], F32)
            bfc = cp.tile([n_graphs, 8], F32)
            for tt, dt in ((iota, t_iota), (eye, t_eye),
                           (sdt, t_sd), (swt, t_sw), (w1, t_w1), (b1c, t_b1),
                           (b2r, t_b2r), (btv, t_bt),
                           (icnt, t_icnt), (wfc, t_wfc), (bfc, t_bfc)):
                nc.sync.dma_start(out=tt[:, :], in_=dt[:, :])
            for k in range(8):  # replicate 16-row idx wrap across 128 partitions
                nc.sync.dma_start(out=gidx[16 * k : 16 * (k + 1), :],
                                  in_=t_gidx[:, :])
            for h in range(nh):
                nc.sync.dma_start(out=w2h[h][:, :],
                                  in_=t_w2[h * 128 : (h + 1) * 128, :])

            # stage shard into internal DRAM (collectives can't read IO), gather full x
            xl = dp.tile([npc, in_dim], TDT)
            nc.sync.dma_start(out=xl[:, :], in_=t_xs[:, :])
            xfull = dp.tile([n_nodes, in_dim], TDT, addr_space="Shared")
            nc.gpsimd.collective_compute(
                "AllGather", mybir.AluOpType.bypass,
                replica_groups=[list(range(NCORES))],
                ins=[xl[:, :].opt()], outs=[xfull[:, :].opt()])
            h2l = dp.tile([npc, oh], TDT)

            # ---------------- layer 1 ----------------
            with (
                tc.tile_pool(name="gp1", bufs=2) as gp,
                tc.tile_pool(name="sp1", bufs=2) as sp,
                tc.tile_pool(name="st1", bufs=3) as stp,
                tc.tile_pool(name="ps_agg1", bufs=2, space="PSUM") as ps_agg,
                tc.tile_pool(name="ps_h1", bufs=2, space="PSUM") as ps_h,
                tc.tile_pool(name="ps_h2", bufs=2, space="PSUM") as ps_h2p,
                tc.tile_pool(name="ps_tr1", bufs=2, space="PSUM") as ps_tr,
            ):
                ch = 0
                for t in range(ntile):
                    ca, cb = int(cpt[t, 0]), int(cpt[t, 1])
                    gA = gp.tile([128, CAM, 128], TDT, tag="gA")
                    gB = gp.tile([128, CBM, 128], TDT, tag="gB")
                    for o in range(0, ca, GCAP):
                        k = min(GCAP, ca - o)
                        nc.gpsimd.dma_gather(
                            gA[:, o : o + k, :], xfull[0:TSPLIT, :],
                            gidx[:, (ch + o) * 8 : (ch + o + k) * 8],
                            k * 128, k * 128, in_dim)
                    for o in range(0, cb, GCAP):
                        k = min(GCAP, cb - o)
                        nc.gpsimd.dma_gather(
                            gB[:, o : o + k, :], xfull[TSPLIT:n_nodes, :],
                            gidx[:, (ch + ca + o) * 8 : (ch + ca + o + k) * 8],
                            k * 128, k * 128, in_dim)
                    pt = ps_agg.tile([128, 128], F32, tag="agg")
                    nck = ca + cb
                    sg = sp.tile([128, CAM + CBM, 128], TDT, tag="s")
                    iota_b = iota[:, :].unsqueeze(1).broadcast_to([128, nck, 128])
                    sd_b = sdt[:, ch : ch + nck].unsqueeze(2).broadcast_to(
                        [128, nck, 128])
                    sw_b = swt[:, ch : ch + nck].unsqueeze(2).broadcast_to(
                        [128, nck, 128])
                    nc.vector.tensor_tensor(out=sg[:, 0:nck, :], in0=iota_b,
                                            in1=sd_b, op=mybir.AluOpType.is_equal)
                    nc.vector.tensor_tensor(out=sg[:, 0:nck, :], in0=sg[:, 0:nck, :],
                                            in1=sw_b, op=mybir.AluOpType.mult)
                    for j in range(nck):
                        g = gA[:, j, :] if j < ca else gB[:, j - ca, :]
                        # feat-major agg^T: out[feat, dstlane]
                        nc.tensor.matmul(pt[:, :], lhsT=g, rhs=sg[:, j, :],
                                         start=(j == 0), stop=(j == nck - 1))
                    ch += nck
                    aggs = stp.tile([128, 128], F32, tag="aggs")
                    nc.vector.tensor_copy(aggs[:, :], pt[:, :])
                    # h1^T = relu(W1^T agg + b1), two 128-halves
                    ph = ps_h.tile([128, hid], F32, tag="ph")
                    h1s = stp.tile([128, hid], F32, tag="h1s")
                    for h in range(nh):
                        nc.tensor.matmul(ph[:, h * 128 : (h + 1) * 128],
                                         lhsT=w1[:, h * 128 : (h + 1) * 128],
                                         rhs=aggs[:, :], start=True, stop=True)
                        nc.scalar.activation(
                            out=h1s[:, h * 128 : (h + 1) * 128],
                            in_=ph[:, h * 128 : (h + 1) * 128],
                            func=mybir.ActivationFunctionType.Relu,
                            bias=b1c[:, h : h + 1], scale=1.0)
                    # h2pre^T = W2^T h1
                    p2 = ps_h2p.tile([128, oh], F32, tag="p2")
                    for h in range(nh):
                        nc.tensor.matmul(p2[:, :], lhsT=w2h[h][:, :],
                                         rhs=h1s[:, h * 128 : (h + 1) * 128],
                                         start=(h == 0), stop=(h == nh - 1))
                    hp = stp.tile([128, oh], F32, tag="hp")
                    nc.vector.tensor_copy(hp[:, :], p2[:, :])
                    ptr = ps_tr.tile([128, 128], F32, tag="tr")
                    nc.tensor.transpose(ptr[:, :], hp[:, :], eye[:, :])
                    ro = stp.tile([128, 128], TDT, tag="ro")
                    nc.vector.tensor_copy(ro[:, :], ptr[:, :])
                    rows = min(128, npc - t * 128)
                    nc.sync.dma_start(out=h2l[t * 128 : t * 128 + rows, :],
                                      in_=ro[0:rows, :])
                    if stage == 1:
                        nc.sync.dma_start(
                            out=t_dbg[t * 128 : (t + 1) * 128, :], in_=ro[:, :])

            if stage < 3:
                with tc.tile_pool(name="zz", bufs=1) as zp:
                    zz = zp.tile([n_graphs, 8], F32)
                    nc.vector.tensor_scalar(out=zz[:, :], in0=bfc[:, :], scalar1=0.0,
                                            scalar2=None, op0=mybir.AluOpType.mult)
                    nc.sync.dma_start(out=t_out[:, :], in_=zz[:, :])

            if stage >= 2:
                _build_l2_section(nc, tc, meta, n_nodes, oh, n_graphs, stage,
                                  dp, cp, locals())
    nc.compile()
    return nc


def _build_l2_section(nc, tc, meta, n_nodes, oh, n_graphs, stage, dp, cp, env):
    npc, ntile, cpt, nch = meta["npc"], meta["ntile"], meta["cpt"], meta["nch"]
    CAM, CBM = int(cpt[:, 0].max()), int(cpt[:, 1].max())
    iota, gidx, sdt, swt = env["iota"], env["gidx"], env["sdt"], env["swt"]
    b2r, btv, icnt, wfc, bfc = (env["b2r"], env["btv"], env["icnt"],
                                env["wfc"], env["bfc"])
    h2l, t_out, t_dbg = env["h2l"], env["t_out"], env["t_dbg"]

    h2full = dp.tile([n_nodes, oh], TDT, addr_space="Shared")
    nc.gpsimd.collective_compute(
        "AllGather", mybir.AluOpType.bypass,
        replica_groups=[list(range(NCORES))],
        ins=[h2l[:, :].opt()], outs=[h2full[:, :].opt()])

    # ---------------- layer 2 + pool ----------------
    with (
                tc.tile_pool(name="gp2", bufs=2) as gp,
                tc.tile_pool(name="sp2", bufs=2) as sp,
                tc.tile_pool(name="st2", bufs=3) as stp,
                tc.tile_pool(name="ps_agg2", bufs=2, space="PSUM") as ps_agg,
                tc.tile_pool(name="ps_pool", bufs=1, space="PSUM") as ps_pool,
                tc.tile_pool(name="ps_fc", bufs=1, space="PSUM") as ps_fc,
            ):
                ppool = ps_pool.tile([128, n_graphs], F32)
                ch = 0
                for t in range(ntile):
                    ca, cb = int(cpt[t, 0]), int(cpt[t, 1])
                    gA = gp.tile([128, CAM, 128], TDT, tag="gA")
                    gB = gp.tile([128, CBM, 128], TDT, tag="gB")
                    for o in range(0, ca, GCAP):
                        k = min(GCAP, ca - o)
                        nc.gpsimd.dma_gather(
                            gA[:, o : o + k, :], h2full[0:TSPLIT, :],
                            gidx[:, (ch + o) * 8 : (ch + o + k) * 8],
                            k * 128, k * 128, oh)
                    for o in range(0, cb, GCAP):
                        k = min(GCAP, cb - o)
                        nc.gpsimd.dma_gather(
                            gB[:, o : o + k, :], h2full[TSPLIT:n_nodes, :],
                            gidx[:, (ch + ca + o) * 8 : (ch + ca + o + k) * 8],
                            k * 128, k * 128, oh)
                    pt = ps_agg.tile([128, 128], F32, tag="agg")
                    nck = ca + cb
                    sg = sp.tile([128, CAM + CBM, 128], TDT, tag="s")
                    iota_b = iota[:, :].unsqueeze(1).broadcast_to([128, nck, 128])
                    sd_b = sdt[:, ch : ch + nck].unsqueeze(2).broadcast_to(
                        [128, nck, 128])
                    sw_b = swt[:, ch : ch + nck].unsqueeze(2).broadcast_to(
                        [128, nck, 128])
                    nc.vector.tensor_tensor(out=sg[:, 0:nck, :], in0=iota_b,
                                            in1=sd_b, op=mybir.AluOpType.is_equal)
                    nc.vector.tensor_tensor(out=sg[:, 0:nck, :], in0=sg[:, 0:nck, :],
                                            in1=sw_b, op=mybir.AluOpType.mult)
                    for j in range(nck):
                        g = gA[:, j, :] if j < ca else gB[:, j - ca, :]
                        # node-major: out[dstlane, feat]
                        nc.tensor.matmul(pt[:, :], lhsT=sg[:, j, :], rhs=g,
                                         start=(j == 0), stop=(j == nck - 1))
                    ch += nck
                    h2 = stp.tile([128, oh], F32, tag="h2")
                    nc.vector.tensor_tensor(out=h2[:, :], in0=pt[:, :],
                                            in1=b2r[:, :], op=mybir.AluOpType.add)
                    nc.vector.tensor_scalar(
                        out=h2[:, :], in0=h2[:, :], scalar1=0.0, scalar2=None,
                        op0=mybir.AluOpType.max)
                    eq = sp.tile([128, n_graphs], F32, tag="eq")
                    nc.vector.tensor_scalar(
                        out=eq[:, :], in0=iota[:, 0:n_graphs],
                        scalar1=btv[:, t : t + 1], scalar2=None,
                        op0=mybir.AluOpType.is_equal)
                    nc.tensor.matmul(ppool[:, :], lhsT=h2[:, :], rhs=eq[:, :],
                                     start=(t == 0), stop=(t == ntile - 1))
                    if stage == 2:
                        nc.sync.dma_start(
                            out=t_dbg[t * 128 : (t + 1) * 128, :], in_=h2[:, :])

                if stage == 2:
                    return
                # per-core partial FC output; host sums partials and adds bfc
                pooled = stp.tile([128, n_graphs], F32, tag="pooled")
                nc.vector.tensor_copy(pooled[:, :], ppool[:, :])
                pfc = ps_fc.tile([n_graphs, 8], F32)
                nc.tensor.matmul(pfc[:, :], lhsT=pooled[:, :], rhs=wfc[:, :],
                                 start=True, stop=True)
                osb = stp.tile([n_graphs, 8], F32, tag="osb")
                nc.vector.tensor_scalar(
                    out=osb[:, :], in0=pfc[:, :], scalar1=icnt[:, 0:1],
                    scalar2=None, op0=mybir.AluOpType.mult)
                nc.sync.dma_start(out=t_out[:, :], in_=osb[:, :])


def kernel(x, src, dst, batch, W1, b1, W2, b2, Wfc, bfc):
    global last_result
    x = np.asarray(x, np.float32)
    src = np.asarray(src, np.int64)
    dst = np.asarray(dst, np.int64)
    batch = np.asarray(batch, np.int64)
    W1, b1v, W2, b2v, Wfc, bfcv = (np.asarray(a, np.float32)
                                   for a in (W1, b1, W2, b2, Wfc, bfc))
    n, in_dim = x.shape
    hid = W1.shape[1]
    oh = W2.shape[1]
    ng = 64
    odim = Wfc.shape[1]

    meta = _plan(src, dst, n)
    npc, ntile, nch = meta["npc"], meta["ntile"], meta["nch"]

    key = (n, in_dim, hid, oh, meta["cpt"].tobytes())
    if key not in _cache:
        _cache[key] = _build(meta, n, in_dim, hid, oh, ng)
    nc = _cache[key]

    iota = np.tile(np.arange(128, dtype=np.float32), (128, 1))
    eye = np.eye(128, dtype=np.float32)
    cntg = np.maximum(np.bincount(batch, minlength=ng).astype(np.float32), 1.0)
    icnt = (1.0 / cntg).reshape(ng, 1)
    b1c = np.ascontiguousarray(b1v.reshape(hid // 128, 128).T)
    b2r = np.tile(b2v.reshape(1, oh), (128, 1)).astype(np.float32)
    wfc8 = np.zeros((oh, 8), np.float32)
    wfc8[:, :odim] = Wfc
    bfc8 = np.zeros((ng, 8), np.float32)
    bfc8[:, :odim] = bfcv.reshape(1, odim)

    ins = []
    for c in range(NCORES):
        iw, sdv, swv = meta["cores"][c]
        bl = np.full(ntile * 128, -1.0, np.float32)
        bl[:npc] = batch[c * npc : (c + 1) * npc].astype(np.float32)
        ins.append({
            "xs": x[c * npc : (c + 1) * npc].astype(ml_dtypes.bfloat16)
                  if TDT == BF16 else x[c * npc : (c + 1) * npc],
            "gidx": iw, "sd": sdv, "sw": swv,
            "w1": W1, "b1c": b1c, "w2": W2, "b2r": b2r,
            "batchv": np.ascontiguousarray(bl.reshape(ntile, 128).T),
            "invcnt": icnt, "wfc": wfc8, "bfc": bfc8,
            "iota": iota, "eye": eye,
        })
    import time as _t
    _s = _t.time()
    r = run_bass_kernel_spmd(nc, ins, core_ids=list(range(NCORES)))
    exec_wall[0] = _t.time() - _s
    last_result = r
    out = np.zeros((ng, 8), np.float32)
    for c in range(NCORES):
        out += np.asarray(r.results[c]["out"], np.float32)
    out += bfc8
    return np.ascontiguousarray(out[:ng, :odim])
